# revision 8
# baseline (speedup 1.0000x reference)
"""Trainium2 Bass kernel for nn_DiscriminationLoss (segment_reduce).

v2 design (8 NeuronCores, pixel-sharded; full inputs in, full loss out):

  - Each core gets 1/8 of the 4M pixels: pred slice [8, 524288] f32 and
    labels slice [524288] i32, tiled [128 partitions x 4096 block-cols].
  - One-hot generation on DVE via per-class tensor_scalar(is_equal, j)
    at 4x perf mode (16-bit dense step-1 APs): 32 ops per 1024-col chunk,
    ~42us/core total (vs ~70us for the tensor_tensor+iota variant: TT is
    capped at 2x; TS reaches 4x_2p).  Each TS op also emits accum_out =
    per-partition sum of its one-hot = per-chunk label counts, so no
    "ones" column is needed in the matmul stationary.
  - pred is scaled by 2^14 and cast f32->fp16 on ScalarE into the
    block-diagonal stationary layout [p, (unit, c, b)] (qb=8 blocks per
    unit, 8 channels -> 64-column stationary slabs).
  - The PE runs TWO concurrent column-tiles (128x64 array mode):
    tile t in {0,1} handles units u with u%2==t,
      psum[64t:64t+64, :256] += slabh_unit[128, 64]^T @ oh_unit[128, 256]
    Each tile streams its own moving operand, so the aggregate moving
    rate is ~2 cols/cycle: ~28us of PE vs ~56us untiled.  Only the 8
    diagonal [8, 32] sub-blocks of each [64, 256] product are meaningful;
    the host extracts them.
  - Labels ride the hardware DMA queue (Sync) as int32, interleaved
    ahead of the pred groups, and are cast int32->bf16 on ScalarE.  (The
    SWDGE cast-DMA used previously is a hidden ~50us serial stream.)
  - Per-kernel counts: one tiny f32 matmul (ones[128,1]^T @ cnt[128,128])
    turns the accum_out partials into per-(chunk, class) counts.
  - Warmup matmuls on a memset tile trip the PE HAM clock gate during
    the otherwise-dead first ~12us (DMA/one-hot pipeline fill).
  - Host sums per-core partials (the "psum" step of the sharding hint)
    and evaluates the tiny O(K^2) pairwise tail in f64.
"""

import sys
import functools

sys.path.insert(0, "/opt/trn_rl_repo")

import numpy as np

C = 8
K = 32
NCORES = 8
H = W = 2048
PTOT = H * W
PCORE = PTOT // NCORES  # 524288
SIGMA_DIS = 3.0
PRED_SCALE = float(2.0**14)

QB = 8            # pixel-blocks per matmul unit (block-diagonal trick)
NSTAT = C * QB    # stationary columns per unit = 64
NMOV = K * QB     # moving columns per unit = 256
WARM_MMS = 96     # PE warmup matmuls (trip the HAM clock gate early)


def _group_sizes(ftot, fg):
    """Pred DMA/cast group sizes (block-cols): small head groups prime the
    pipeline, small tail groups shorten the exposed DMA tail."""
    if ftot >= 8 * fg:
        gs = [fg // 4, 3 * fg // 4] + [fg] * (ftot // fg - 2) + [3 * fg // 4, fg // 4]
    else:
        gs = [fg] * (ftot // fg)
    assert sum(gs) == ftot
    return gs


def _chunk_sizes(ftot, fc):
    if ftot <= fc:
        return [ftot]
    return [fc] * (ftot // fc)


def build_nc(pcore=PCORE, fg=512, fc=1024, warm=WARM_MMS):
    import concourse.bacc as bacc
    import concourse.tile as tile
    import concourse.mybir as mybir
    from contextlib import ExitStack

    assert pcore % 128 == 0
    ftot = pcore // 128
    fg = min(fg, ftot)
    fc = min(fc, ftot)
    assert ftot % fg == 0 and ftot % fc == 0
    gs = _group_sizes(ftot, fg)
    cs = _chunk_sizes(ftot, fc)
    nchunks = len(cs)
    nunits = ftot // QB

    f32 = mybir.dt.float32
    bf16 = mybir.dt.bfloat16
    fp16 = mybir.dt.float16
    i32 = mybir.dt.int32

    nc = bacc.Bacc(
        "TRN2", target_bir_lowering=False, debug=False, num_devices=NCORES
    )
    pred_ext = nc.dram_tensor("pred", [C, pcore], f32, kind="ExternalInput")
    lab_ext = nc.dram_tensor("labels", [pcore], i32, kind="ExternalInput")
    out_ext = nc.dram_tensor("out_s", [128, NMOV], f32, kind="ExternalOutput")
    # row 0: counts per (chunk, class); row 32: warm dump (keeps warm MMs live)
    aux_ext = nc.dram_tensor("out_aux", [40, K * nchunks], f32, kind="ExternalOutput")

    with tile.TileContext(nc) as tc, ExitStack() as ctx:
        const_pool = ctx.enter_context(tc.tile_pool(name="const", bufs=1))
        lab32_pool = ctx.enter_context(tc.tile_pool(name="lab32", bufs=3))
        labbf_pool = ctx.enter_context(tc.tile_pool(name="labbf", bufs=1))
        slab32_pool = ctx.enter_context(tc.tile_pool(name="slab32", bufs=2))
        slabh_pool = ctx.enter_context(tc.tile_pool(name="slabh", bufs=3))
        oh_pool = ctx.enter_context(tc.tile_pool(name="oh", bufs=2))
        psum_pool = ctx.enter_context(tc.tile_pool(name="psum", bufs=1, space="PSUM"))
        out_pool = ctx.enter_context(tc.tile_pool(name="outp", bufs=1))

        # constants / scratch
        warm_t = const_pool.tile([128, 128], bf16)
        nc.gpsimd.memset(warm_t[:], 1.0)
        ones_f32 = const_pool.tile([128, 1], f32)
        nc.gpsimd.memset(ones_f32[:], 1.0)
        cnt = const_pool.tile([128, K * nchunks], f32)
        labbf = labbf_pool.tile([128, ftot], bf16)
        outaux = out_pool.tile([128, K * nchunks], f32)
        nc.gpsimd.memset(outaux[:40, :], 0.0)

        psum_s = psum_pool.tile([128, NMOV], f32)
        cnt_ps = psum_pool.tile([128, K * nchunks], f32)
        warm_ps = psum_pool.tile([128, 128], f32)

        # PE warmup on memset data: runs during the head DMA/one-hot fill.
        if warm:
            for w in range(warm):
                nc.tensor.matmul(
                    warm_ps[:64, :],
                    warm_t[:, :64],
                    warm_t[:, :128],
                    start=(w == 0),
                    stop=(w == warm - 1),
                )

        # ---- DMA + cast streams -------------------------------------------
        # Sync-queue order: first three label chunks lead (the one-hot
        # pipeline starts on labels only), then pred groups interleave.
        lab_dmas = []
        goff = 0
        for g, fgg in enumerate(gs):
            lt = lab32_pool.tile([128, fg], i32, tag="lab32")
            lab_dmas.append((lt, goff, fgg))
            goff += fgg

        def issue_label_dma(g):
            lt, off, fgg = lab_dmas[g]
            nc.sync.dma_start(
                lt[:, :fgg],
                lab_ext[128 * off : 128 * (off + fgg)].rearrange(
                    "(p f) -> p f", p=128
                ),
            )

        def cast_labels(g):
            lt, off, fgg = lab_dmas[g]
            nc.scalar.activation(
                labbf[:, off : off + fgg],
                lt[:, :fgg],
                mybir.ActivationFunctionType.Copy,
            )

        slabhs = []  # per group: (tile, unit_start)
        issue_label_dma(0)
        issue_label_dma(1)
        issue_label_dma(2)
        cast_labels(0)
        cast_labels(1)
        cast_labels(2)
        goff = 0
        for g, fgg in enumerate(gs):
            if g + 3 < len(gs):
                issue_label_dma(g + 3)
                cast_labels(g + 3)
            gpx = 128 * fgg
            poff = 128 * goff
            slab32 = slab32_pool.tile([128, C * fg], f32, tag="slab32")
            s32 = slab32[:, : C * fgg]
            nc.sync.dma_start(
                s32.rearrange("p (c f) -> p c f", c=C),
                pred_ext[:, poff : poff + gpx].rearrange("c (p f) -> p c f", p=128),
            )
            # slabh layout: [p, (u, c, b)] — each unit's stationary
            # [128, 64] is a contiguous slice.
            slabh = slabh_pool.tile([128, QB * C * (fg // QB)], fp16, tag="slabh")
            slabh_r = slabh[:, : C * fgg].rearrange(
                "p (u c b) -> p u c b", c=C, b=QB
            )
            slab32_r = s32.rearrange("p (c u b) -> p u c b", c=C, b=QB)
            nc.scalar.activation(
                slabh_r,
                slab32_r,
                mybir.ActivationFunctionType.Copy,
                scale=PRED_SCALE,
            )
            slabhs.append((slabh, goff // QB))
            goff += fgg

        # unit -> (slabh tile, within-group unit index)
        unit_map = []
        for (sh, ustart), fgg in zip(slabhs, gs):
            for ug in range(fgg // QB):
                unit_map.append((sh, ug))
        assert len(unit_map) == nunits

        # ---- one-hot chunks + matmuls -------------------------------------
        u = 0
        coff = 0
        for ci, fcc in enumerate(cs):
            ucount = fcc // QB
            oh = oh_pool.tile([128, K * fc], fp16, tag="oh")
            oh_r = oh[:, : K * fcc].rearrange("p (u j b) -> p u j b", j=K, b=QB)
            in0 = labbf[:, coff : coff + fcc].rearrange("p (u b) -> p u b", b=QB)
            for j in range(K):
                nc.vector.tensor_scalar(
                    oh_r[:, :, j, :],
                    in0,
                    float(j + 1),
                    None,
                    mybir.AluOpType.is_equal,
                    mybir.AluOpType.add,
                    accum_out=cnt[:, ci * K + j : ci * K + j + 1],
                )
            for uc in range(ucount):
                sh, ug = unit_map[u]
                t = u % 2
                nc.tensor.matmul(
                    psum_s[64 * t : 64 * t + 64, :],
                    sh[:, ug * NSTAT : (ug + 1) * NSTAT],
                    oh[:, uc * NMOV : (uc + 1) * NMOV],
                    start=(u < 2),
                    stop=(u >= nunits - 2),
                    tile_position=(0, 64 * t),
                    skip_group_check=True,
                )
                u += 1
            coff += fcc

        # ---- counts matmul + output ---------------------------------------
        nc.tensor.matmul(
            cnt_ps[:1, :],
            ones_f32[:],
            cnt[:],
            start=True,
            stop=True,
        )
        outt = out_pool.tile([128, NMOV], f32)
        nc.vector.tensor_copy(outt[:], psum_s[:])
        nc.vector.tensor_copy(outaux[:1, :], cnt_ps[:1, :])
        if warm:
            nc.vector.tensor_copy(
                outaux[32:33, : min(128, K * nchunks)],
                warm_ps[32:33, : min(128, K * nchunks)],
            )
        nc.sync.dma_start(out_ext[:], outt[:])
        nc.sync.dma_start(aux_ext[:], outaux[:40, :])
    nc.compile()
    return nc


@functools.lru_cache(maxsize=1)
def _get_program():
    return build_nc()


def make_in_maps(pred_flat, labels_flat, pcore=PCORE, ncores=NCORES):
    in_maps = []
    for i in range(ncores):
        sl = slice(i * pcore, (i + 1) * pcore)
        in_maps.append(
            {
                "pred": np.ascontiguousarray(pred_flat[:, sl]),
                "labels": np.ascontiguousarray(labels_flat[sl]),
            }
        )
    return in_maps


def extract_SN(res_core, nchunks):
    """From one core's outputs: S_scaled [C, K] and N [K]."""
    ps = res_core["out_s"].astype(np.float64)  # [128, 256]
    aux = res_core["out_aux"].astype(np.float64)  # [8, K*nchunks]
    S = np.zeros((C, K))
    for t in range(2):
        r = ps[64 * t : 64 * t + 64, :].reshape(C, QB, K, QB)
        S += r[:, np.arange(QB), :, np.arange(QB)].sum(axis=0)
    N = aux[0].reshape(nchunks, K).sum(axis=0)
    return S, N


def finish_host(results, num_kernel, nchunks=4):
    S = np.zeros((C, K))
    N = np.zeros(K)
    for r in results:
        Si, Ni = extract_SN(r, nchunks)
        S += Si
        N += Ni
    S /= PRED_SCALE
    A = N * np.sum(S * S, axis=0)  # [K]
    kk = int(num_kernel)
    A = A[:kk]
    pair = A[:, None] + A[None, :]
    Dm = np.maximum(SIGMA_DIS - np.sqrt(pair), 0.0)
    term = np.log(Dm * Dm + 1.0)
    L = float(np.sum(np.triu(term, k=1)))
    L *= (kk - 1) / kk
    return np.float32(L)


_last_results = None


def kernel(pred_similarities, regions_mask, kernel_labels, num_kernel, **kw):
    global _last_results
    from concourse.bass_utils import run_bass_kernel_spmd

    pred_flat = np.asarray(pred_similarities, dtype=np.float32).reshape(C, PTOT)
    labels_flat = np.asarray(kernel_labels, dtype=np.int32).reshape(PTOT)

    nc = _get_program()
    in_maps = make_in_maps(pred_flat, labels_flat)
    res = run_bass_kernel_spmd(nc, in_maps, list(range(NCORES)))
    _last_results = res
    return finish_host(
        [res.results[i] for i in range(NCORES)], num_kernel
    )


# revision 19
# speedup vs baseline: 2.1202x; 2.1202x over previous
"""Trainium2 Bass kernel for nn_DiscriminationLoss (segment_reduce).

v2 design (8 NeuronCores, pixel-sharded; full inputs in, full loss out):

  - Each core gets 1/8 of the 4M pixels: pred slice [8, 524288] f32 and
    labels slice [524288] i32, tiled [128 partitions x 4096 block-cols].
  - One-hot generation on DVE via per-class tensor_scalar(is_equal, j)
    at 4x perf mode (16-bit dense step-1 APs): 32 ops per 1024-col chunk,
    ~42us/core total (vs ~70us for the tensor_tensor+iota variant: TT is
    capped at 2x; TS reaches 4x_2p).  Each TS op also emits accum_out =
    per-partition sum of its one-hot = per-chunk label counts, so no
    "ones" column is needed in the matmul stationary.
  - pred is scaled by 2^14 and cast f32->fp16 on ScalarE into the
    block-diagonal stationary layout [p, (unit, c, b)] (qb=8 blocks per
    unit, 8 channels -> 64-column stationary slabs).
  - The PE runs TWO concurrent column-tiles (128x64 array mode):
    tile t in {0,1} handles units u with u%2==t,
      psum[64t:64t+64, :256] += slabh_unit[128, 64]^T @ oh_unit[128, 256]
    Each tile streams its own moving operand, so the aggregate moving
    rate is ~2 cols/cycle: ~28us of PE vs ~56us untiled.  Only the 8
    diagonal [8, 32] sub-blocks of each [64, 256] product are meaningful;
    the host extracts them.
  - Labels ride the hardware DMA queue (Sync) as int32, interleaved
    ahead of the pred groups, and are cast int32->bf16 on ScalarE.  (The
    SWDGE cast-DMA used previously is a hidden ~50us serial stream.)
  - Per-kernel counts ride the stationary's 9th (ones) column — the
    36-col stationary leaves room in the 64-col tile, so no accum_out
    (whose CACHE_REDUCE lowering runs at 1x) and no extra streams.
  - Warmup matmuls on a memset tile trip the PE HAM clock gate during
    the otherwise-dead first ~12us (DMA/one-hot pipeline fill).
  - Host sums per-core partials (the "psum" step of the sharding hint)
    and evaluates the tiny O(K^2) pairwise tail in f64.
"""

import sys
import functools

sys.path.insert(0, "/opt/trn_rl_repo")

import numpy as np

C = 8
K = 32
NCORES = 8
H = W = 2048
PTOT = H * W
PCORE = PTOT // NCORES  # 524288
SIGMA_DIS = 3.0
PRED_SCALE = float(2.0**14)

QB = 4            # pixel-blocks per matmul unit (block-diagonal trick)
NCH = C + 1       # 8 pred channels + ones column (counts)
NSTAT = NCH * QB  # stationary columns per unit = 36 (fits the 64-col tile)
NMOV = K * QB     # moving columns per unit = 128
WARM_MMS = 96     # PE warmup matmuls (trip the HAM clock gate early)


def _group_sizes(ftot, fg):
    """Pred DMA/cast group sizes (block-cols): small head groups prime the
    pipeline, small tail groups shorten the exposed DMA tail."""
    if ftot >= 8 * fg:
        gs = [fg // 4, 3 * fg // 4] + [fg] * (ftot // fg - 2) + [3 * fg // 4, fg // 4]
    else:
        gs = [fg] * (ftot // fg)
    assert sum(gs) == ftot
    return gs


def _chunk_sizes(ftot, fc):
    if ftot <= fc:
        return [ftot]
    return [fc] * (ftot // fc)


def build_nc(pcore=PCORE, fg=512, fc=1024, warm=WARM_MMS):
    import concourse.bacc as bacc
    import concourse.tile as tile
    import concourse.mybir as mybir
    from contextlib import ExitStack

    assert pcore % 128 == 0
    ftot = pcore // 128
    fg = min(fg, ftot)
    fc = min(fc, ftot)
    assert ftot % fg == 0 and ftot % fc == 0
    gs = _group_sizes(ftot, fg)
    cs = _chunk_sizes(ftot, fc)
    nchunks = len(cs)
    nunits = ftot // QB

    f32 = mybir.dt.float32
    bf16 = mybir.dt.bfloat16
    fp16 = mybir.dt.float16
    i32 = mybir.dt.int32

    nc = bacc.Bacc(
        "TRN2", target_bir_lowering=False, debug=False, num_devices=NCORES
    )
    pred_ext = nc.dram_tensor("pred", [C, pcore], f32, kind="ExternalInput")
    lab_ext = nc.dram_tensor("labels", [pcore], i32, kind="ExternalInput")
    out_ext = nc.dram_tensor("out_s", [128, NMOV], f32, kind="ExternalOutput")
    # row 32: warm dump (keeps warm MMs live)
    aux_ext = nc.dram_tensor("out_aux", [40, 128], f32, kind="ExternalOutput")

    with tile.TileContext(nc) as tc, ExitStack() as ctx:
        const_pool = ctx.enter_context(tc.tile_pool(name="const", bufs=1))
        lab32_pool = ctx.enter_context(tc.tile_pool(name="lab32", bufs=3))
        labbf_pool = ctx.enter_context(tc.tile_pool(name="labbf", bufs=1))
        slab32_pool = ctx.enter_context(tc.tile_pool(name="slab32", bufs=2))
        slabh_pool = ctx.enter_context(tc.tile_pool(name="slabh", bufs=3))
        oh_pool = ctx.enter_context(tc.tile_pool(name="oh", bufs=2))
        psum_pool = ctx.enter_context(tc.tile_pool(name="psum", bufs=1, space="PSUM"))
        out_pool = ctx.enter_context(tc.tile_pool(name="outp", bufs=1))

        # constants / scratch
        warm_t = const_pool.tile([128, 128], bf16)
        nc.gpsimd.memset(warm_t[:], 1.0)
        ones_f32 = const_pool.tile([128, 1], f32)
        nc.gpsimd.memset(ones_f32[:], 1.0)
        labbf = labbf_pool.tile([128, ftot], bf16)
        outaux = out_pool.tile([128, 128], f32)
        nc.gpsimd.memset(outaux[:40, :], 0.0)

        psum_s = psum_pool.tile([128, NMOV], f32)
        warm_ps = psum_pool.tile([128, 128], f32)

        # PE warmup on memset data: runs during the head DMA/one-hot fill.
        if warm:
            for w in range(warm):
                nc.tensor.matmul(
                    warm_ps[:64, :],
                    warm_t[:, :64],
                    warm_t[:, :128],
                    start=(w == 0),
                    stop=(w == warm - 1),
                )

        # ---- DMA + cast streams -------------------------------------------
        # Sync-queue order: first three label chunks lead (the one-hot
        # pipeline starts on labels only), then pred groups interleave.
        lab_dmas = []
        goff = 0
        for g, fgg in enumerate(gs):
            lt = lab32_pool.tile([128, fg], i32, tag="lab32")
            lab_dmas.append((lt, goff, fgg))
            goff += fgg

        def issue_label_dma(g):
            lt, off, fgg = lab_dmas[g]
            nc.sync.dma_start(
                lt[:, :fgg],
                lab_ext[128 * off : 128 * (off + fgg)].rearrange(
                    "(p f) -> p f", p=128
                ),
            )

        def cast_labels(g):
            lt, off, fgg = lab_dmas[g]
            nc.scalar.activation(
                labbf[:, off : off + fgg],
                lt[:, :fgg],
                mybir.ActivationFunctionType.Copy,
            )

        slabhs = []  # per group: (tile, unit_start)
        issue_label_dma(0)
        issue_label_dma(1)
        issue_label_dma(2)
        cast_labels(0)
        cast_labels(1)
        cast_labels(2)
        goff = 0
        for g, fgg in enumerate(gs):
            if g + 3 < len(gs):
                issue_label_dma(g + 3)
                cast_labels(g + 3)
            gpx = 128 * fgg
            poff = 128 * goff
            slab32 = slab32_pool.tile([128, C * fg], f32, tag="slab32")
            s32 = slab32[:, : C * fgg]
            nc.sync.dma_start(
                s32.rearrange("p (c f) -> p c f", c=C),
                pred_ext[:, poff : poff + gpx].rearrange("c (p f) -> p c f", p=128),
            )
            # slabh layout: [p, (u, c, b)] with c in 0..8 (8 pred channels +
            # ones column) — each unit's stationary [128, 36] is a
            # contiguous slice.
            slabh = slabh_pool.tile([128, NSTAT * (fg // QB)], fp16, tag="slabh")
            slabh_r = slabh[:, : NSTAT * fgg // QB].rearrange(
                "p (u c b) -> p u c b", c=NCH, b=QB
            )
            slab32_r = s32.rearrange("p (c u b) -> p u c b", c=C, b=QB)
            nc.scalar.activation(
                slabh_r[:, :, :C, :],
                slab32_r,
                mybir.ActivationFunctionType.Copy,
                scale=PRED_SCALE,
            )
            # ones column via ACT: Copy(0*x + 1) = 1.0 (input always ready)
            nc.scalar.activation(
                slabh_r[:, :, C, :],
                ones_f32[:, :1]
                .unsqueeze(2)
                .broadcast_to([128, fgg // QB, QB]),
                mybir.ActivationFunctionType.Copy,
                bias=1.0,
                scale=0.0,
            )
            slabhs.append((slabh, goff // QB))
            goff += fgg

        # unit -> (slabh tile, within-group unit index)
        unit_map = []
        for (sh, ustart), fgg in zip(slabhs, gs):
            for ug in range(fgg // QB):
                unit_map.append((sh, ug))
        assert len(unit_map) == nunits

        # ---- one-hot chunks + matmuls -------------------------------------
        u = 0
        coff = 0
        for ci, fcc in enumerate(cs):
            ucount = fcc // QB
            oh = oh_pool.tile([128, K * fc], fp16, tag="oh")
            oh_r = oh[:, : K * fcc].rearrange("p (u j b) -> p u j b", j=K, b=QB)
            in0 = labbf[:, coff : coff + fcc].rearrange("p (u b) -> p u b", b=QB)
            for j in range(K):
                nc.vector.tensor_scalar(
                    oh_r[:, :, j, :],
                    in0,
                    float(j + 1),
                    None,
                    mybir.AluOpType.is_equal,
                )
            for uc in range(ucount):
                sh, ug = unit_map[u]
                t = u % 2
                nc.tensor.matmul(
                    psum_s[64 * t : 64 * t + NSTAT, :],
                    sh[:, ug * NSTAT : (ug + 1) * NSTAT],
                    oh[:, uc * NMOV : (uc + 1) * NMOV],
                    start=(u < 2),
                    stop=(u >= nunits - 2),
                    tile_position=(0, 64 * t),
                    skip_group_check=True,
                )
                u += 1
            coff += fcc

        # ---- output --------------------------------------------------------
        outt = out_pool.tile([128, NMOV], f32)
        nc.vector.memset(outt[:], 0.0)
        nc.vector.tensor_copy(outt[:NSTAT, :], psum_s[:NSTAT, :])
        nc.vector.tensor_copy(
            outt[64 : 64 + NSTAT, :], psum_s[64 : 64 + NSTAT, :]
        )
        if warm:
            nc.vector.tensor_copy(outaux[32:33, :], warm_ps[32:33, :])
        nc.sync.dma_start(out_ext[:], outt[:])
        nc.sync.dma_start(aux_ext[:], outaux[:40, :])
    nc.compile()
    return nc


@functools.lru_cache(maxsize=1)
def _get_program():
    return build_nc()


def make_in_maps(pred_flat, labels_flat, pcore=PCORE, ncores=NCORES):
    in_maps = []
    for i in range(ncores):
        sl = slice(i * pcore, (i + 1) * pcore)
        in_maps.append(
            {
                "pred": np.ascontiguousarray(pred_flat[:, sl]),
                "labels": np.ascontiguousarray(labels_flat[sl]),
            }
        )
    return in_maps


def extract_SN(res_core):
    """From one core's outputs: S_scaled [C, K] and N [K]."""
    ps = res_core["out_s"].astype(np.float64)  # [128, NMOV]
    S = np.zeros((C, K))
    N = np.zeros(K)
    for t in range(2):
        r = ps[64 * t : 64 * t + NSTAT, :].reshape(NCH, QB, K, QB)
        d = r[:, np.arange(QB), :, np.arange(QB)].sum(axis=0)  # [NCH, K]
        S += d[:C, :]
        N += d[C, :]
    return S, N


def finish_host(results, num_kernel):
    S = np.zeros((C, K))
    N = np.zeros(K)
    for r in results:
        Si, Ni = extract_SN(r)
        S += Si
        N += Ni
    S /= PRED_SCALE
    A = N * np.sum(S * S, axis=0)  # [K]
    kk = int(num_kernel)
    A = A[:kk]
    pair = A[:, None] + A[None, :]
    Dm = np.maximum(SIGMA_DIS - np.sqrt(pair), 0.0)
    term = np.log(Dm * Dm + 1.0)
    L = float(np.sum(np.triu(term, k=1)))
    L *= (kk - 1) / kk
    return np.float32(L)


_last_results = None


def kernel(pred_similarities, regions_mask, kernel_labels, num_kernel, **kw):
    global _last_results
    from concourse.bass_utils import run_bass_kernel_spmd

    pred_flat = np.asarray(pred_similarities, dtype=np.float32).reshape(C, PTOT)
    labels_flat = np.asarray(kernel_labels, dtype=np.int32).reshape(PTOT)

    nc = _get_program()
    in_maps = make_in_maps(pred_flat, labels_flat)
    res = run_bass_kernel_spmd(nc, in_maps, list(range(NCORES)))
    _last_results = res
    return finish_host(
        [res.results[i] for i in range(NCORES)], num_kernel
    )
